# revision 9
# baseline (speedup 1.0000x reference)
"""Causal self-attention (B=2,T=2048,C=1024,H=16,hd=64) with QK-RMSNorm + RoPE.

SINGLE-CORE Trainium2 Bass kernel. Measured dispatch costs in this
environment: ~0.24ms per bound tensor per 8-core dispatch and a multi-device
sync floor of ~2-5ms, while a 1-core dispatch with 2 bound tensors and 25MB
of I/O costs ~0.2-0.5ms total and on-device DMA/compute is comparatively
free. So the fastest configuration runs the whole layer on ONE core with ONE
packed input tensor and one fp16 output tensor.

Structure (per 512-token block n, head-pair group g of 2 heads):
  - QKV matmuls from bf16 x tiles and bf16 w_attn (feature-major, host
    pre-transposed); RMS-norm stats from raw PSUM; RoPE via host-permuted
    [evens, odds] feature order (swap halves by SBUF DMA); q/k stored fp16
    (q in SBUF for the current blocks, k resident in SBUF [128, 8, 4096]).
  - V transposed to token-major via PE transpose, staged to DRAM with an
    appended ones column per head (softmax denominator; |s|<=8 so exp needs
    no max subtraction).
  - attention per (batch, query-block, g): scores from lhsT=k, rhs=q with
    tile_position packing two 64-wide contractions per PSUM tile; exp on
    the scalar engine; causal wedge on gpsimd; AV from DRAM-streamed V
    tiles; denominator normalize; y (bf16) staged to DRAM.
  - c_proj: per 128-token tile, accumulate 8 head-group matmuls
    (lhsT=y-tile, rhs=w_proj.T tile) -> [128 tokens, 1024] PSUM -> fp16 out.
"""

import numpy as np

import concourse.bass as bass
import concourse.mybir as mybir
import concourse.tile as tile
from concourse import bacc
from concourse.bass_utils import run_bass_kernel_spmd

B, T, C = 2, 2048, 1024
H, HD = 16, 64
N_CORES = 1
NG = 8            # head-pair groups (2 heads -> 128 feature rows each)
BT = B * T
EPS = 1e-6
TN = BT // 512    # 8 token blocks of 512
QB = T // 512     # 4 query blocks per sequence

# packed single-input blob layout (bf16 columns, [128, NB])
OX = 0                    # x, p-major per block: [8 blocks][128, 8*512]
OWA = OX + TN * 4096      # w_attn sel.T p-major [128, 8*3072]
OWP = OWA + 8 * 3072      # w_proj.T p-major [128, 8*1024]
OCS = OWP + 8 * 1024      # rows 0:32 cs f32, rows 32:64 sn f32 [32, 2048]
OQW = OCS + 4096          # [128, 1] f32
OKW = OQW + 2
OBO = OKW + 2             # bones [128, 2] f32
OS2 = OBO + 4             # sel2 [2, 128] f32
OWG = OS2 + 256           # wedge [128, 128] f32
OID = OWG + 256           # ident [128, 128] f32
OVO = OID + 256           # vones [128, 32] f32
NB = OVO + 64

f32 = mybir.dt.float32
f32r = mybir.dt.float32r
f16 = mybir.dt.float16
bf16 = mybir.dt.bfloat16
MUL = mybir.AluOpType.mult
ADD = mybir.AluOpType.add
AF = mybir.ActivationFunctionType


def r32(ap):
    return ap.bitcast(f32r)


def build_nc():
    nc = bacc.Bacc("TRN2", target_bir_lowering=False, debug=False,
                   num_devices=1)

    blob = nc.dram_tensor("blob", [128, NB], bf16, kind="ExternalInput")
    out = nc.dram_tensor("out", [BT, C], f16, kind="ExternalOutput")

    bap = blob.ap()
    wa_ap = bap[:, OWA:OWA + 8 * 3072].rearrange("p (o f) -> p o f", o=8)
    wp_ap = bap[:, OWP:OWP + 8 * 1024].rearrange("p (o f) -> p o f", o=8)
    cs_ap = bap[0:32, OCS:OCS + 4096].bitcast(f32)
    sn_ap = bap[32:64, OCS:OCS + 4096].bitcast(f32)
    qw_ap = bap[:, OQW:OQW + 2].bitcast(f32)
    kw_ap = bap[:, OKW:OKW + 2].bitcast(f32)
    bo_ap = bap[:, OBO:OBO + 4].bitcast(f32r)
    s2_ap = bap[0:2, OS2:OS2 + 256].bitcast(f32r)
    wg_ap = bap[:, OWG:OWG + 256].bitcast(f32)
    id_ap = bap[:, OID:OID + 256].bitcast(f32)
    vo_ap = bap[:, OVO:OVO + 64].bitcast(f32)

    with tile.TileContext(nc) as tc:
        with (
            tc.tile_pool(name="const", bufs=1) as const,
            tc.tile_pool(name="resid", bufs=1) as resid,
            tc.tile_pool(name="xtp", bufs=3) as xtp,
            tc.tile_pool(name="qsp", bufs=2) as qsp,
            tc.tile_pool(name="work", bufs=3) as work,
            tc.tile_pool(name="pwork", bufs=6) as pwork,
            tc.tile_pool(name="vtp", bufs=2) as vtp,
            tc.tile_pool(name="ybp", bufs=2) as ybp,
            tc.tile_pool(name="ktp", bufs=2) as ktp,
            tc.tile_pool(name="mm", bufs=2, space="PSUM") as mmp,
            tc.tile_pool(name="yp", bufs=2, space="PSUM") as ypp,
            tc.tile_pool(name="sp", bufs=1, space="PSUM") as spp,
            tc.tile_pool(name="bcp", bufs=1, space="PSUM") as bcp,
            tc.tile_pool(name="dram", bufs=1, space="DRAM") as dramp,
        ):
            # ---- DRAM staging tensors ----
            # V token-major per group: [NG, 128 tok-part, 32 ktg, 2*(HD+1)]
            # per-head stride HD+2=66: col 64 = ones (denominator), col 65
            # = ones padding (16-bit matmul operands need even widths)
            VW = HD + 2
            vdr = dramp.tile([NG, 128, BT // 128, 2 * VW], f16, tag="vdr")
            # attention output per group: [NG, 128 feat, BT] bf16
            ydr = dramp.tile([NG, 128, BT], bf16, tag="ydr")
            # k, roped+normed, fp16: [NG, 128 rows = 2 heads feat, keys]
            kdr = dramp.tile([NG, 128, BT], f16, tag="kdr")

            # ---- constants to SBUF ----
            wa_sb = const.tile([128, 8, 3 * C], bf16, tag="wa")
            nc.sync.dma_start(wa_sb[:], wa_ap)
            qw_sb = const.tile([128, 1], f32, tag="qw")
            nc.sync.dma_start(qw_sb[:], qw_ap)
            kw_sb = const.tile([128, 1], f32, tag="kw")
            nc.sync.dma_start(kw_sb[:], kw_ap)
            bo_sb = const.tile([128, 2], f32r, tag="bo")
            nc.sync.dma_start(bo_sb[:], bo_ap)
            s2_sb = const.tile([2, 128], f32r, tag="s2")
            nc.sync.dma_start(s2_sb[:], s2_ap)
            id_sb = const.tile([128, 128], f32, tag="id")
            nc.sync.dma_start(id_sb[:], id_ap)
            eps_sb = const.tile([128, 1], f32, tag="eps")
            nc.vector.memset(eps_sb[:], EPS)
            cs_sb = const.tile([128, T], f32, tag="cs")
            sn_sb = const.tile([128, T], f32, tag="sn")
            wg_sb = const.tile([128, 128], f32, tag="wg")
            on_sb = const.tile([128, 64], f16, tag="on")
            wp_sb = resid.tile([128, 8, C], bf16, tag="wp_sb")

            def emit_late_consts():
                nc.vector.memset(on_sb[:], 1.0)
                for b0 in (0, 32, 64, 96):
                    nc.sync.dma_start(cs_sb[b0:b0 + 32, :], cs_ap)
                    nc.sync.dma_start(sn_sb[b0:b0 + 32, :], sn_ap)
                # sign pattern [-sn, sn, -sn, sn] built in place
                nc.scalar.mul(sn_sb[0:32, :], sn_sb[0:32, :], -1.0)
                nc.scalar.mul(sn_sb[64:96, :], sn_sb[64:96, :], -1.0)
                nc.sync.dma_start(wg_sb[:], wg_ap)
                nc.sync.dma_start(wp_sb[:], wp_ap)
                # ones columns of V (denominator source), once per (g, h)
                for g in range(NG):
                    for h in range(2):
                        nc.sync.dma_start(
                            vdr[g][:, :, VW * h + HD:VW * h + HD + 2],
                            on_sb[:, :].rearrange("p (a b) -> p a b", b=2))


            # ================= QKV + RMSNorm + RoPE =================
            xts = {}

            def emit_xt(n):
                xt = xtp.tile([128, 8, 512], bf16, tag="xt", name=f"xt{n}")
                nc.sync.dma_start(
                    xt[:], bap[:, OX + 4096 * n:OX + 4096 * (n + 1)]
                    .rearrange("p (o t) -> p o t", o=8))
                xts[n] = xt

            def emit_qkv(n, qtiles):
                tok = slice(512 * n, 512 * n + 512)
                ct = slice(512 * (n % QB), 512 * (n % QB) + 512)
                if n not in xts:
                    emit_xt(n)
                xt = xts.pop(n)
                qsb = qsp.tile([128, NG, 512], f16, tag="qsb", name=f"qsb{n}")
                qtiles[n] = qsb

                for g in range(NG):
                    bigQK = mmp.tile([128, 1024], f32, tag="big",
                                     name=f"qk{n}_{g}")
                    bigV = mmp.tile([128, 1024], f32, tag="big",
                                    name=f"v{n}_{g}")
                    for m, wcol in ((0, qw_sb), (1, kw_sb), (2, None)):
                        ps = (bigV[:, 0:512] if m == 2
                              else bigQK[:, 512 * m:512 * m + 512])
                        wsl = slice(C * m + 128 * g, C * m + 128 * g + 128)
                        for kt in range(C // 128):
                            nc.tensor.matmul(
                                ps, wa_sb[:, kt, wsl], xt[:, kt, :],
                                start=(kt == 0), stop=(kt == C // 128 - 1),
                            )
                        if m == 2:
                            # V: token-major via PE transpose of 128x128 blocks
                            vs = work.tile([128, 512], f32, tag="vs",
                                           name=f"vs{n}_{g}")
                            nc.vector.tensor_copy(vs[:], ps)
                            vtm = vtp.tile([128, 4, 2, HD], f16, tag="vtm",
                                           name=f"vtm{n}_{g}")
                            for j in range(4):
                                pt = spp.tile([128, 128], f32, tag="sm",
                                              name=f"vt{n}_{g}_{j}")
                                nc.tensor.transpose(
                                    pt[:], vs[:, 128 * j:128 * j + 128],
                                    id_sb[:])
                                nc.vector.tensor_copy(
                                    vtm[:, j],
                                    pt[:, :].rearrange("p (h d) -> p h d", h=2))
                            for h in range(2):
                                nc.sync.dma_start(
                                    vdr[g][:, 4 * n:4 * n + 4,
                                           VW * h:VW * h + HD],
                                    vtm[:, :, h, :])
                            continue

                        # stats from raw (pre-weight) psum
                        sq = work.tile([128, 512], f32, tag="scr",
                                       name=f"sq{n}_{g}_{m}")
                        nc.scalar.activation(r32(sq[:]), ps, AF.Square)
                        ss = spp.tile([2, 512], f32, tag="sm",
                                      name=f"ss{n}_{g}_{m}")
                        nc.tensor.matmul(ss[:], r32(bo_sb[:]), r32(sq[:]),
                                         start=True, stop=True)
                        inv = work.tile([2, 512], f32, tag="rms",
                                        name=f"rms{n}_{g}_{m}")
                        nc.scalar.activation(r32(inv[:]), ss[:], AF.Sqrt,
                                             bias=eps_sb[0:2, :],
                                             scale=1.0 / HD)
                        with nc.allow_low_precision(reason="f32r is fp32-width"):
                            nc.vector.reciprocal(r32(inv[:]), inv[:])

                        # norm weight on the way out of PSUM (f32 scratch)
                        qt = work.tile([128, 512], f32, tag="qt",
                                       name=f"qt{n}_{g}_{m}")
                        nc.vector.tensor_scalar_mul(r32(qt[:]), ps, wcol[:])

                        # rope: r = q*CS + swap(q)*SN (swap halves per head)
                        sw = work.tile([128, 512], f32r, tag="sw",
                                       name=f"sw{n}_{g}_{m}")
                        for h in range(2):
                            b0 = 64 * h
                            nc.sync.dma_start(sw[b0:b0 + 32, :],
                                              r32(qt[b0 + 32:b0 + 64, :]))
                            nc.sync.dma_start(sw[b0 + 32:b0 + 64, :],
                                              r32(qt[b0:b0 + 32, :]))
                        nc.gpsimd.tensor_tensor(sw[:], sw[:], sn_sb[:, ct],
                                                MUL)
                        nc.vector.tensor_tensor(r32(qt[:]), qt[:],
                                                cs_sb[:, ct], MUL)
                        nc.vector.tensor_tensor(r32(qt[:]), qt[:], sw[:], ADD)

                        # 1/rms: broadcast [2,512] -> [128,512] via K=2 matmul
                        bc = bcp.tile([128, 512], f32, tag="bc",
                                      name=f"bc{n}_{g}_{m}")
                        nc.tensor.matmul(bc[:], r32(s2_sb[:]), r32(inv[:]),
                                         start=True, stop=True)
                        if m == 0:
                            nc.vector.tensor_tensor(qsb[:, g, :], qt[:],
                                                    bc[:], MUL)
                        else:
                            kst = ktp.tile([128, 512], f16, tag="kst",
                                           name=f"kst{n}_{g}")
                            nc.vector.tensor_tensor(kst[:], qt[:], bc[:],
                                                    MUL)
                            nc.sync.dma_start(kdr[g][:, tok], kst[:])

            # ================= causal attention =================
            def emit_attn2(b, i, qtiles):
                n = QB * b + i
                qsb = qtiles.pop(n)
                nkt = 4 * i + 4
                qcol = slice(2048 * b + 512 * i, 2048 * b + 512 * i + 512)
                for g in range(NG):
                    yps = [ypp.tile([HD + 2, 512], f32, tag="y",
                                    name=f"y{b}_{i}_{g}_{h}")
                           for h in range(2)]
                    vts = vtp.tile([128, 16, 2 * VW], f16, tag="vts",
                                   name=f"vts{b}_{i}_{g}")
                    nc.sync.dma_start(vts[:, 0:nkt, :],
                                      vdr[g][:, 16 * b:16 * b + nkt, :])
                    kts = ktp.tile([128, 2048], f16, tag="kts",
                                   name=f"kts{b}_{i}_{g}")
                    nc.sync.dma_start(
                        kts[:, 0:128 * nkt],
                        kdr[g][:, 2048 * b:2048 * b + 128 * nkt])
                    def emit_av(kt, pT):
                        qs = 128 * (kt - 4 * i) if kt >= 4 * i else 0
                        for h in range(2):
                            nc.tensor.matmul(
                                yps[h][:, qs:],
                                vts[:, kt, VW * h:VW * (h + 1)],
                                pT[:, 512 * h + qs:512 * h + 512],
                                start=(kt == 0), stop=(kt == nkt - 1),
                            )

                    pending = None
                    for kt in range(nkt):
                        qs = 128 * (kt - 4 * i) if kt >= 4 * i else 0
                        sps = mmp.tile([128, 1024], f32, tag="big",
                                       name=f"s{b}_{i}_{g}_{kt}")
                        pT = pwork.tile([128, 1024], f16, tag="pT",
                                        name=f"p{b}_{i}_{g}_{kt}")
                        for h in range(2):
                            hb = 64 * h
                            nc.tensor.matmul(
                                sps[:, 512 * h + qs:512 * h + 512],
                                kts[hb:hb + 64, 128 * kt:128 * kt + 128],
                                qsb[hb:hb + 64, g, qs:],
                                start=True, stop=True,
                                tile_position=(hb, 0),
                            )
                        sps3 = sps[:, :].rearrange("p (h q) -> p h q",
                                                   h=2)[:, :, qs:]
                        pT3 = pT[:, :].rearrange("p (h q) -> p h q",
                                                 h=2)[:, :, qs:]
                        nc.scalar.activation(pT3, sps3, AF.Exp,
                                             scale=1.0 / 8.0)
                        for h in range(2):
                            if kt >= 4 * i:
                                nc.gpsimd.tensor_tensor(
                                    pT[:, 512 * h + qs:512 * h + qs + 128],
                                    pT[:, 512 * h + qs:512 * h + qs + 128],
                                    wg_sb[:], MUL)
                        if pending is not None:
                            emit_av(*pending)
                        pending = (kt, pT)
                    emit_av(*pending)
                    # normalize by the ones-column denominator
                    for h in range(2):
                        di = work.tile([1, 512], f32, tag="rms",
                                       name=f"di{b}_{i}_{g}_{h}")
                        with nc.allow_low_precision(reason="f32r is fp32-width"):
                            nc.vector.reciprocal(r32(di[:]),
                                                 yps[h][HD:HD + 1, :])
                        dp = spp.tile([64, 512], f32, tag="sm",
                                      name=f"dp{b}_{i}_{g}_{h}")
                        nc.tensor.matmul(dp[:], r32(s2_sb[0:1, 0:64]),
                                         r32(di[:]), start=True, stop=True)
                        dpS = work.tile([64, 512], f32, tag="dpS",
                                        name=f"dpS{b}_{i}_{g}_{h}")
                        nc.vector.tensor_copy(dpS[:], dp[:])
                        ybf = ybp.tile([HD, 512], bf16, tag="ybf",
                                       name=f"ybf{b}_{i}_{g}_{h}")
                        nc.vector.tensor_tensor(ybf[:, :], yps[h][:HD, :],
                                                dpS[:, :], MUL)
                        nc.sync.dma_start(ydr[g][64 * h:64 * h + HD, qcol],
                                          ybf[:, :])

            def emit_cproj(tt):
                ybr = ybp.tile([128, 8, 128], bf16, tag="ybr",
                               name=f"ybr{tt}")
                nc.sync.dma_start(
                    ybr[:], ydr[:, :, 128 * tt:128 * tt + 128]
                    .rearrange("g p t -> p g t"))
                po = mmp.tile([128, 1024], f32, tag="big", name=f"po{tt}")
                for cc in range(2):
                    for g in range(NG):
                        nc.tensor.matmul(
                            po[:, 512 * cc:512 * cc + 512], ybr[:, g, :],
                            wp_sb[:, g, 512 * cc:512 * cc + 512],
                            start=(g == 0), stop=(g == NG - 1),
                        )
                ob = work.tile([128, 1024], f16, tag="obf", name=f"ob{tt}")
                nc.vector.tensor_copy(ob[:], po[:])
                nc.sync.dma_start(out[128 * tt:128 * tt + 128, :], ob[:])

            qtiles = {}
            emit_xt(0)
            emit_late_consts()
            for n in range(TN):
                emit_qkv(n, qtiles)
                b, i = divmod(n, QB)
                emit_attn2(b, i, qtiles)
                if n >= 1:
                    # c_proj for the previous attention's 512-token range
                    for tt in range(4 * (n - 1), 4 * n):
                        emit_cproj(tt)
            for tt in range(4 * (TN - 1), 4 * TN):
                emit_cproj(tt)



    nc.compile()
    return nc


def make_in_maps(x, freqs_cos, freqs_sin, w_attn, w_proj, q_norm_w, k_norm_w):
    x = np.asarray(x, np.float32)
    freqs_cos = np.asarray(freqs_cos, np.float32)
    freqs_sin = np.asarray(freqs_sin, np.float32)
    w_attn = np.asarray(w_attn, np.float32)
    w_proj = np.asarray(w_proj, np.float32)
    q_norm_w = np.asarray(q_norm_w, np.float32)
    k_norm_w = np.asarray(k_norm_w, np.float32)

    perm = np.concatenate([np.arange(0, HD, 2), np.arange(1, HD, 2)])
    import ml_dtypes
    bfloat16 = ml_dtypes.bfloat16
    xTf = np.ascontiguousarray(x.reshape(BT, C).T.astype(bfloat16))
    wpT = w_proj.T.astype(bfloat16)  # [1024 in-feat, 1024 out-feat]

    cs32 = np.ascontiguousarray(freqs_cos.T).astype(np.float32)  # [32, T]
    sn32 = np.ascontiguousarray(freqs_sin.T).astype(np.float32)

    qwc = np.tile(q_norm_w[perm], 2)[:, None].astype(np.float32)
    kwc = np.tile(k_norm_w[perm], 2)[:, None].astype(np.float32)

    bones = np.zeros((128, 2), np.float32)
    bones[:64, 0] = 1.0
    bones[64:, 1] = 1.0
    sel2 = np.zeros((2, 128), np.float32)
    sel2[0, :64] = 1.0
    sel2[1, 64:] = 1.0
    wedge = (np.arange(128)[:, None] <= np.arange(128)[None, :]).astype(
        np.float32)
    vones = np.ones((128, 32), np.float32)
    ident = np.eye(128, dtype=np.float32)

    # w_attn rows: per section, all 16 heads; q/k rows permuted [evens, odds]
    rows = []
    for sec in range(3):
        for h in range(H):
            base = C * sec + HD * h
            if sec < 2:
                rows.append(base + perm)
            else:
                rows.append(base + np.arange(HD))
    sel_rows = np.concatenate(rows)
    waT = np.ascontiguousarray(w_attn[sel_rows].T.astype(bfloat16))
    # [1024, 3072]; col = sec*1024 + 128*g + local

    bb = np.zeros((128, NB), bfloat16)

    def putf32(arr, r0, c0):
        v = np.ascontiguousarray(arr.astype(np.float32)).view(bfloat16)
        bb[r0:r0 + v.shape[0], c0:c0 + v.shape[1]] = v

    for n in range(TN):
        bb[:, OX + 4096 * n:OX + 4096 * (n + 1)] = (
            xTf[:, 512 * n:512 * n + 512]
            .reshape(8, 128, 512).transpose(1, 0, 2).reshape(128, 4096))
    bb[:, OWA:OWA + 8 * 3072] = (
        waT.reshape(8, 128, 3072).transpose(1, 0, 2).reshape(128, 8 * 3072))
    bb[:, OWP:OWP + 8 * 1024] = (
        np.ascontiguousarray(wpT)
        .reshape(8, 128, 1024).transpose(1, 0, 2).reshape(128, 8 * 1024))
    putf32(cs32, 0, OCS)
    putf32(sn32, 32, OCS)
    putf32(qwc, 0, OQW)
    putf32(kwc, 0, OKW)
    putf32(bones, 0, OBO)
    putf32(sel2, 0, OS2)
    putf32(wedge, 0, OWG)
    putf32(ident, 0, OID)
    putf32(vones, 0, OVO)
    return [{"blob": bb}]


_NC_CACHE = {}


def get_nc():
    if "nc" not in _NC_CACHE:
        _NC_CACHE["nc"] = build_nc()
    return _NC_CACHE["nc"]


def kernel(x, freqs_cos, freqs_sin, w_attn, w_proj, q_norm_w, k_norm_w):
    nc = get_nc()
    in_maps = make_in_maps(x, freqs_cos, freqs_sin, w_attn, w_proj,
                           q_norm_w, k_norm_w)
    res = run_bass_kernel_spmd(nc, in_maps, core_ids=[0])
    return (res.results[0]["out"].reshape(B, T, C).astype(np.float32))


# revision 11
# speedup vs baseline: 1.1503x; 1.1503x over previous
"""Causal self-attention (B=2,T=2048,C=1024,H=16,hd=64) with QK-RMSNorm + RoPE.

SINGLE-CORE Trainium2 Bass kernel. Measured dispatch costs in this
environment: ~0.24ms per bound tensor per 8-core dispatch and a multi-device
sync floor of ~2-5ms, while a 1-core dispatch with 2 bound tensors and 25MB
of I/O costs ~0.2-0.5ms total and on-device DMA/compute is comparatively
free. So the fastest configuration runs the whole layer on ONE core with ONE
packed input tensor and one fp16 output tensor.

Structure (per 512-token block n, head-pair group g of 2 heads):
  - QKV matmuls from bf16 x tiles and bf16 w_attn (feature-major, host
    pre-transposed); RMS-norm stats from raw PSUM; RoPE via host-permuted
    [evens, odds] feature order (swap halves by SBUF DMA); q/k stored fp16
    (q in SBUF for the current blocks, k resident in SBUF [128, 8, 4096]).
  - V transposed to token-major via PE transpose, staged to DRAM with an
    appended ones column per head (softmax denominator; |s|<=8 so exp needs
    no max subtraction).
  - attention per (batch, query-block, g): scores from lhsT=k, rhs=q with
    tile_position packing two 64-wide contractions per PSUM tile; exp on
    the scalar engine; causal wedge on gpsimd; AV from DRAM-streamed V
    tiles; denominator normalize; y (bf16) staged to DRAM.
  - c_proj: per 128-token tile, accumulate 8 head-group matmuls
    (lhsT=y-tile, rhs=w_proj.T tile) -> [128 tokens, 1024] PSUM -> fp16 out.
"""

import numpy as np

import concourse.bass as bass
import concourse.mybir as mybir
import concourse.tile as tile
from concourse import bacc
from concourse.bass_utils import run_bass_kernel_spmd

B, T, C = 2, 2048, 1024
H, HD = 16, 64
N_CORES = 1
NG = 8            # head-pair groups (2 heads -> 128 feature rows each)
BT = B * T
EPS = 1e-6
TN = BT // 512    # 8 token blocks of 512
QB = T // 512     # 4 query blocks per sequence

# packed single-input blob layout (bf16 columns, [128, NB])
OX = 0                    # x, p-major per block: [8 blocks][128, 8*512]
OWA = OX + TN * 4096      # w_attn sel.T p-major [128, 8*3072]
OWP = OWA + 8 * 3072      # w_proj.T p-major [128, 8*1024]
OCS = OWP + 8 * 1024      # rows 0:32 cs f32, rows 32:64 sn f32 [32, 2048]
OQW = OCS + 4096          # [128, 1] f32
OKW = OQW + 2
OBO = OKW + 2             # bones [128, 2] f32
OS2 = OBO + 4             # sel2 [2, 128] f32
OWG = OS2 + 256           # wedge [128, 128] f32
OID = OWG + 256           # ident [128, 128] f32
OVO = OID + 256           # vones [128, 32] f32
NB = OVO + 64

f32 = mybir.dt.float32
f32r = mybir.dt.float32r
f16 = mybir.dt.float16
bf16 = mybir.dt.bfloat16
MUL = mybir.AluOpType.mult
ADD = mybir.AluOpType.add
AF = mybir.ActivationFunctionType


def r32(ap):
    return ap.bitcast(f32r)


def build_nc():
    nc = bacc.Bacc("TRN2", target_bir_lowering=False, debug=False,
                   num_devices=1)

    blob = nc.dram_tensor("blob", [128, NB], bf16, kind="ExternalInput")
    out = nc.dram_tensor("out", [BT, C], f16, kind="ExternalOutput")

    bap = blob.ap()
    wa_ap = bap[:, OWA:OWA + 8 * 3072].rearrange("p (o f) -> p o f", o=8)
    wp_ap = bap[:, OWP:OWP + 8 * 1024].rearrange("p (o f) -> p o f", o=8)
    cs_ap = bap[0:32, OCS:OCS + 4096].bitcast(f32)
    sn_ap = bap[32:64, OCS:OCS + 4096].bitcast(f32)
    qw_ap = bap[:, OQW:OQW + 2].bitcast(f32)
    kw_ap = bap[:, OKW:OKW + 2].bitcast(f32)
    bo_ap = bap[:, OBO:OBO + 4].bitcast(f32r)
    s2_ap = bap[0:2, OS2:OS2 + 256].bitcast(f32r)
    wg_ap = bap[:, OWG:OWG + 256].bitcast(f32)
    id_ap = bap[:, OID:OID + 256].bitcast(f32)
    vo_ap = bap[:, OVO:OVO + 64].bitcast(f32)

    with tile.TileContext(nc) as tc:
        with (
            tc.tile_pool(name="const", bufs=1) as const,
            tc.tile_pool(name="resid", bufs=1) as resid,
            tc.tile_pool(name="xtp", bufs=3) as xtp,
            tc.tile_pool(name="qsp", bufs=2) as qsp,
            tc.tile_pool(name="work", bufs=3) as work,
            tc.tile_pool(name="pwork", bufs=6) as pwork,
            tc.tile_pool(name="vtp", bufs=2) as vtp,
            tc.tile_pool(name="ybp", bufs=2) as ybp,
            tc.tile_pool(name="ktp", bufs=2) as ktp,
            tc.tile_pool(name="mm", bufs=2, space="PSUM") as mmp,
            tc.tile_pool(name="yp", bufs=2, space="PSUM") as ypp,
            tc.tile_pool(name="sp", bufs=1, space="PSUM") as spp,
            tc.tile_pool(name="bcp", bufs=1, space="PSUM") as bcp,
            tc.tile_pool(name="dram", bufs=1, space="DRAM") as dramp,
        ):
            # ---- DRAM staging tensors ----
            # V token-major per group: [NG, 128 tok-part, 32 ktg, 2*(HD+1)]
            # per-head stride HD+2=66: col 64 = ones (denominator), col 65
            # = ones padding (16-bit matmul operands need even widths)
            VW = HD + 2
            vdr = dramp.tile([NG, 128, BT // 128, 2 * VW], f16, tag="vdr")
            # attention output per group: [NG, 128 feat, BT] bf16
            ydr = dramp.tile([NG, 128, BT], bf16, tag="ydr")
            # k, roped+normed, fp16: [NG, 128 rows = 2 heads feat, keys]
            kdr = dramp.tile([NG, 128, BT], f16, tag="kdr")

            # ---- constants to SBUF ----
            wa_sb = const.tile([128, 8, 3 * C], bf16, tag="wa")
            nc.sync.dma_start(wa_sb[:], wa_ap)
            qw_sb = const.tile([128, 1], f32, tag="qw")
            nc.sync.dma_start(qw_sb[:], qw_ap)
            kw_sb = const.tile([128, 1], f32, tag="kw")
            nc.sync.dma_start(kw_sb[:], kw_ap)
            bo_sb = const.tile([128, 2], f32r, tag="bo")
            nc.sync.dma_start(bo_sb[:], bo_ap)
            s2_sb = const.tile([2, 128], f32r, tag="s2")
            nc.sync.dma_start(s2_sb[:], s2_ap)
            id_sb = const.tile([128, 128], f32, tag="id")
            nc.sync.dma_start(id_sb[:], id_ap)
            eps_sb = const.tile([128, 1], f32, tag="eps")
            nc.vector.memset(eps_sb[:], EPS)
            cs_sb = const.tile([128, T], f32, tag="cs")
            sn_sb = const.tile([128, T], f32, tag="sn")
            wg_sb = const.tile([128, 128], f32, tag="wg")
            on_sb = const.tile([128, 64], f16, tag="on")
            wp_sb = resid.tile([128, 8, C], bf16, tag="wp_sb")

            def emit_late_consts():
                nc.vector.memset(on_sb[:], 1.0)
                for b0 in (0, 32, 64, 96):
                    nc.sync.dma_start(cs_sb[b0:b0 + 32, :], cs_ap)
                    nc.sync.dma_start(sn_sb[b0:b0 + 32, :], sn_ap)
                # sign pattern [-sn, sn, -sn, sn] built in place
                nc.scalar.mul(sn_sb[0:32, :], sn_sb[0:32, :], -1.0)
                nc.scalar.mul(sn_sb[64:96, :], sn_sb[64:96, :], -1.0)
                nc.sync.dma_start(wg_sb[:], wg_ap)
                nc.sync.dma_start(wp_sb[:], wp_ap)
                # ones columns of V (denominator source), once per (g, h)
                for g in range(NG):
                    for h in range(2):
                        nc.sync.dma_start(
                            vdr[g][:, :, VW * h + HD:VW * h + HD + 2],
                            on_sb[:, :].rearrange("p (a b) -> p a b", b=2))


            # ================= QKV + RMSNorm + RoPE =================
            xts = {}

            def emit_xt(n):
                xt = xtp.tile([128, 8, 512], bf16, tag="xt", name=f"xt{n}")
                nc.sync.dma_start(
                    xt[:], bap[:, OX + 4096 * n:OX + 4096 * (n + 1)]
                    .rearrange("p (o t) -> p o t", o=8))
                xts[n] = xt

            def emit_qkv(n, qtiles):
                tok = slice(512 * n, 512 * n + 512)
                ct = slice(512 * (n % QB), 512 * (n % QB) + 512)
                if n not in xts:
                    emit_xt(n)
                xt = xts.pop(n)
                qsb = qsp.tile([128, NG, 512], f16, tag="qsb", name=f"qsb{n}")
                qtiles[n] = qsb

                for g in range(NG):
                    bigQK = mmp.tile([128, 1024], f32, tag="big",
                                     name=f"qk{n}_{g}")
                    bigV = mmp.tile([128, 1024], f32, tag="big",
                                    name=f"v{n}_{g}")
                    for m, wcol in ((0, qw_sb), (1, kw_sb), (2, None)):
                        ps = (bigV[:, 0:512] if m == 2
                              else bigQK[:, 512 * m:512 * m + 512])
                        wsl = slice(C * m + 128 * g, C * m + 128 * g + 128)
                        for kt in range(C // 128):
                            nc.tensor.matmul(
                                ps, wa_sb[:, kt, wsl], xt[:, kt, :],
                                start=(kt == 0), stop=(kt == C // 128 - 1),
                            )
                        if m == 2:
                            # V: token-major via PE transpose of 128x128 blocks
                            vs = work.tile([128, 512], f32, tag="vs",
                                           name=f"vs{n}_{g}")
                            nc.vector.tensor_copy(vs[:], ps)
                            vtm = vtp.tile([128, 4, 2, HD], f16, tag="vtm",
                                           name=f"vtm{n}_{g}")
                            for j in range(4):
                                pt = spp.tile([128, 128], f32, tag="sm",
                                              name=f"vt{n}_{g}_{j}")
                                nc.tensor.transpose(
                                    pt[:], vs[:, 128 * j:128 * j + 128],
                                    id_sb[:])
                                nc.vector.tensor_copy(
                                    vtm[:, j],
                                    pt[:, :].rearrange("p (h d) -> p h d", h=2))
                            for h in range(2):
                                nc.sync.dma_start(
                                    vdr[g][:, 4 * n:4 * n + 4,
                                           VW * h:VW * h + HD],
                                    vtm[:, :, h, :])
                            continue

                        # stats from raw (pre-weight) psum
                        sq = work.tile([128, 512], f32, tag="scr",
                                       name=f"sq{n}_{g}_{m}")
                        nc.vector.tensor_copy(r32(sq[:]), ps)
                        with nc.allow_low_precision(reason="f32r width"):
                            nc.vector.tensor_tensor(r32(sq[:]), r32(sq[:]),
                                                    r32(sq[:]), MUL)
                        ss = spp.tile([2, 512], f32, tag="sm",
                                      name=f"ss{n}_{g}_{m}")
                        nc.tensor.matmul(ss[:], r32(bo_sb[:]), r32(sq[:]),
                                         start=True, stop=True)
                        inv = work.tile([2, 512], f32, tag="rms",
                                        name=f"rms{n}_{g}_{m}")
                        nc.scalar.activation(r32(inv[:]), ss[:], AF.Sqrt,
                                             bias=eps_sb[0:2, :],
                                             scale=1.0 / HD)
                        with nc.allow_low_precision(reason="f32r is fp32-width"):
                            nc.vector.reciprocal(r32(inv[:]), inv[:])

                        # norm weight on the way out of PSUM (f32 scratch)
                        qt = work.tile([128, 512], f32, tag="qt",
                                       name=f"qt{n}_{g}_{m}")
                        nc.vector.tensor_scalar_mul(r32(qt[:]), ps, wcol[:])

                        # rope: r = q*CS + swap(q)*SN (swap halves per head)
                        sw = work.tile([128, 512], f32r, tag="sw",
                                       name=f"sw{n}_{g}_{m}")
                        for h in range(2):
                            b0 = 64 * h
                            nc.sync.dma_start(sw[b0:b0 + 32, :],
                                              r32(qt[b0 + 32:b0 + 64, :]))
                            nc.sync.dma_start(sw[b0 + 32:b0 + 64, :],
                                              r32(qt[b0:b0 + 32, :]))
                        nc.gpsimd.tensor_tensor(sw[:], sw[:], sn_sb[:, ct],
                                                MUL)
                        nc.vector.tensor_tensor(r32(qt[:]), qt[:],
                                                cs_sb[:, ct], MUL)
                        nc.vector.tensor_tensor(r32(qt[:]), qt[:], sw[:], ADD)

                        # 1/rms: broadcast [2,512] -> [128,512] via K=2 matmul
                        bc = bcp.tile([128, 512], f32, tag="bc",
                                      name=f"bc{n}_{g}_{m}")
                        nc.tensor.matmul(bc[:], r32(s2_sb[:]), r32(inv[:]),
                                         start=True, stop=True)
                        if m == 0:
                            nc.vector.tensor_tensor(qsb[:, g, :], qt[:],
                                                    bc[:], MUL)
                        else:
                            kst = ktp.tile([128, 512], f16, tag="kst",
                                           name=f"kst{n}_{g}")
                            nc.vector.tensor_tensor(kst[:], qt[:], bc[:],
                                                    MUL)
                            nc.sync.dma_start(kdr[g][:, tok], kst[:])

            # ================= causal attention =================
            def emit_attn2(b, i, qtiles):
                n = QB * b + i
                qsb = qtiles.pop(n)
                nkt = 4 * i + 4
                qcol = slice(2048 * b + 512 * i, 2048 * b + 512 * i + 512)
                for g in range(NG):
                    yps = [ypp.tile([HD + 2, 512], f32, tag="y",
                                    name=f"y{b}_{i}_{g}_{h}")
                           for h in range(2)]
                    vts = vtp.tile([128, 16, 2 * VW], f16, tag="vts",
                                   name=f"vts{b}_{i}_{g}")
                    nc.sync.dma_start(vts[:, 0:nkt, :],
                                      vdr[g][:, 16 * b:16 * b + nkt, :])
                    kts = ktp.tile([128, 2048], f16, tag="kts",
                                   name=f"kts{b}_{i}_{g}")
                    nc.sync.dma_start(
                        kts[:, 0:128 * nkt],
                        kdr[g][:, 2048 * b:2048 * b + 128 * nkt])
                    def emit_av(kt, pT):
                        qs = 128 * (kt - 4 * i) if kt >= 4 * i else 0
                        for h in range(2):
                            nc.tensor.matmul(
                                yps[h][:, qs:],
                                vts[:, kt, VW * h:VW * (h + 1)],
                                pT[:, 512 * h + qs:512 * h + 512],
                                start=(kt == 0), stop=(kt == nkt - 1),
                            )

                    pending = []
                    for kt in range(nkt):
                        qs = 128 * (kt - 4 * i) if kt >= 4 * i else 0
                        sps = mmp.tile([128, 1024], f32, tag="big",
                                       name=f"s{b}_{i}_{g}_{kt}")
                        pT = pwork.tile([128, 1024], f16, tag="pT",
                                        name=f"p{b}_{i}_{g}_{kt}")
                        for h in range(2):
                            hb = 64 * h
                            nc.tensor.matmul(
                                sps[:, 512 * h + qs:512 * h + 512],
                                kts[hb:hb + 64, 128 * kt:128 * kt + 128],
                                qsb[hb:hb + 64, g, qs:],
                                start=True, stop=True,
                                tile_position=(hb, 0),
                            )
                        sps3 = sps[:, :].rearrange("p (h q) -> p h q",
                                                   h=2)[:, :, qs:]
                        pT3 = pT[:, :].rearrange("p (h q) -> p h q",
                                                 h=2)[:, :, qs:]
                        nc.scalar.activation(pT3, sps3, AF.Exp,
                                             scale=1.0 / 8.0)
                        for h in range(2):
                            if kt >= 4 * i:
                                nc.gpsimd.tensor_tensor(
                                    pT[:, 512 * h + qs:512 * h + qs + 128],
                                    pT[:, 512 * h + qs:512 * h + qs + 128],
                                    wg_sb[:], MUL)
                        pending.append((kt, pT))
                        if len(pending) > 2:
                            emit_av(*pending.pop(0))
                    for pp in pending:
                        emit_av(*pp)
                    # normalize by the ones-column denominator
                    for h in range(2):
                        di = work.tile([1, 512], f32, tag="rms",
                                       name=f"di{b}_{i}_{g}_{h}")
                        with nc.allow_low_precision(reason="f32r is fp32-width"):
                            nc.vector.reciprocal(r32(di[:]),
                                                 yps[h][HD:HD + 1, :])
                        dp = spp.tile([64, 512], f32, tag="sm",
                                      name=f"dp{b}_{i}_{g}_{h}")
                        nc.tensor.matmul(dp[:], r32(s2_sb[0:1, 0:64]),
                                         r32(di[:]), start=True, stop=True)
                        dpS = work.tile([64, 512], f32, tag="dpS",
                                        name=f"dpS{b}_{i}_{g}_{h}")
                        nc.vector.tensor_copy(dpS[:], dp[:])
                        ybf = ybp.tile([HD, 512], bf16, tag="ybf",
                                       name=f"ybf{b}_{i}_{g}_{h}")
                        nc.vector.tensor_tensor(ybf[:, :], yps[h][:HD, :],
                                                dpS[:, :], MUL)
                        nc.sync.dma_start(ydr[g][64 * h:64 * h + HD, qcol],
                                          ybf[:, :])

            def emit_cproj(tt):
                ybr = ybp.tile([128, 8, 128], bf16, tag="ybr",
                               name=f"ybr{tt}")
                nc.sync.dma_start(
                    ybr[:], ydr[:, :, 128 * tt:128 * tt + 128]
                    .rearrange("g p t -> p g t"))
                po = mmp.tile([128, 1024], f32, tag="big", name=f"po{tt}")
                for cc in range(2):
                    for g in range(NG):
                        nc.tensor.matmul(
                            po[:, 512 * cc:512 * cc + 512], ybr[:, g, :],
                            wp_sb[:, g, 512 * cc:512 * cc + 512],
                            start=(g == 0), stop=(g == NG - 1),
                        )
                ob = work.tile([128, 1024], f16, tag="obf", name=f"ob{tt}")
                nc.vector.tensor_copy(ob[:], po[:])
                nc.sync.dma_start(out[128 * tt:128 * tt + 128, :], ob[:])

            qtiles = {}
            emit_xt(0)
            emit_late_consts()
            for n in range(TN):
                emit_qkv(n, qtiles)
                b, i = divmod(n, QB)
                emit_attn2(b, i, qtiles)
                if n >= 1:
                    # c_proj for the previous attention's 512-token range
                    for tt in range(4 * (n - 1), 4 * n):
                        emit_cproj(tt)
            for tt in range(4 * (TN - 1), 4 * TN):
                emit_cproj(tt)



    nc.compile()
    return nc


def make_in_maps(x, freqs_cos, freqs_sin, w_attn, w_proj, q_norm_w, k_norm_w):
    x = np.asarray(x, np.float32)
    freqs_cos = np.asarray(freqs_cos, np.float32)
    freqs_sin = np.asarray(freqs_sin, np.float32)
    w_attn = np.asarray(w_attn, np.float32)
    w_proj = np.asarray(w_proj, np.float32)
    q_norm_w = np.asarray(q_norm_w, np.float32)
    k_norm_w = np.asarray(k_norm_w, np.float32)

    perm = np.concatenate([np.arange(0, HD, 2), np.arange(1, HD, 2)])
    import ml_dtypes
    bfloat16 = ml_dtypes.bfloat16
    xTf = np.ascontiguousarray(x.reshape(BT, C).T.astype(bfloat16))
    wpT = w_proj.T.astype(bfloat16)  # [1024 in-feat, 1024 out-feat]

    cs32 = np.ascontiguousarray(freqs_cos.T).astype(np.float32)  # [32, T]
    sn32 = np.ascontiguousarray(freqs_sin.T).astype(np.float32)

    qwc = np.tile(q_norm_w[perm], 2)[:, None].astype(np.float32)
    kwc = np.tile(k_norm_w[perm], 2)[:, None].astype(np.float32)

    bones = np.zeros((128, 2), np.float32)
    bones[:64, 0] = 1.0
    bones[64:, 1] = 1.0
    sel2 = np.zeros((2, 128), np.float32)
    sel2[0, :64] = 1.0
    sel2[1, 64:] = 1.0
    wedge = (np.arange(128)[:, None] <= np.arange(128)[None, :]).astype(
        np.float32)
    vones = np.ones((128, 32), np.float32)
    ident = np.eye(128, dtype=np.float32)

    # w_attn rows: per section, all 16 heads; q/k rows permuted [evens, odds]
    rows = []
    for sec in range(3):
        for h in range(H):
            base = C * sec + HD * h
            if sec < 2:
                rows.append(base + perm)
            else:
                rows.append(base + np.arange(HD))
    sel_rows = np.concatenate(rows)
    waT = np.ascontiguousarray(w_attn[sel_rows].T.astype(bfloat16))
    # [1024, 3072]; col = sec*1024 + 128*g + local

    bb = np.zeros((128, NB), bfloat16)

    def putf32(arr, r0, c0):
        v = np.ascontiguousarray(arr.astype(np.float32)).view(bfloat16)
        bb[r0:r0 + v.shape[0], c0:c0 + v.shape[1]] = v

    for n in range(TN):
        bb[:, OX + 4096 * n:OX + 4096 * (n + 1)] = (
            xTf[:, 512 * n:512 * n + 512]
            .reshape(8, 128, 512).transpose(1, 0, 2).reshape(128, 4096))
    bb[:, OWA:OWA + 8 * 3072] = (
        waT.reshape(8, 128, 3072).transpose(1, 0, 2).reshape(128, 8 * 3072))
    bb[:, OWP:OWP + 8 * 1024] = (
        np.ascontiguousarray(wpT)
        .reshape(8, 128, 1024).transpose(1, 0, 2).reshape(128, 8 * 1024))
    putf32(cs32, 0, OCS)
    putf32(sn32, 32, OCS)
    putf32(qwc, 0, OQW)
    putf32(kwc, 0, OKW)
    putf32(bones, 0, OBO)
    putf32(sel2, 0, OS2)
    putf32(wedge, 0, OWG)
    putf32(ident, 0, OID)
    putf32(vones, 0, OVO)
    return [{"blob": bb}]


_NC_CACHE = {}


def get_nc():
    if "nc" not in _NC_CACHE:
        _NC_CACHE["nc"] = build_nc()
    return _NC_CACHE["nc"]


def kernel(x, freqs_cos, freqs_sin, w_attn, w_proj, q_norm_w, k_norm_w):
    nc = get_nc()
    in_maps = make_in_maps(x, freqs_cos, freqs_sin, w_attn, w_proj,
                           q_norm_w, k_norm_w)
    res = run_bass_kernel_spmd(nc, in_maps, core_ids=[0])
    return (res.results[0]["out"].reshape(B, T, C).astype(np.float32))
